# revision 19
# baseline (speedup 1.0000x reference)
"""Trainium2 Bass kernel for masked multi-head attention (8-core SPMD).

Problem: B=2, S=2048, d_in=hid=512, H=8 heads (dh=64), fp32 in/out.
Reference quirk: the mask uses np.tile(valid_length, H), so scores row
i = b*H + h is masked with valid_length[(b*H + h) % 2] = vl[h % 2] —
the mask depends on HEAD PARITY, not batch. Even heads use vl[0], odd
heads vl[1], in both batches.

Sharding (8 cores): core c = (batch b = c//4, head-pair p = c%4).
Each core computes heads {2p, 2p+1} of batch b over the full 2048
queries, producing its partial output [2048, 512] (through its 128
rows of Wo). Host sums the 4 pair-partials per batch (pure unshard).
Load is balanced by construction: every core has one even (long mask)
and one odd (short mask) head.

Perf design (fp16 data path, fp32 PSUM accumulation):
  - All operands stream through the PE at 1 cycle/row (fp32 is 4x
    slower); host pre-converts inputs to fp16, halving HBM traffic.
  - V is projected TRANSPOSED (stationary = 128-key tile of value^T,
    moving = Wv columns) so v_aug [keys, dh] needs no PE transposes.
  - Masking is folded into v_aug: invalid key rows (>= vl, boundary
    tile only) are zeroed INCLUDING the ones-column, so they add 0 to
    both the PV numerator and the softmax denominator — no exp bias,
    which lets exp run on [128, 1024] paired tiles (halves ACT
    instruction overhead). exp(score/8) <= e^~9, safe in fp16.
  - The vaug ones-block is 64 columns wide, so the PV matmul emits
    the softmax denominator already replicated across 64 PSUM
    partitions: normalization is copy + reciprocal_approx_fast +
    multiply, all plain DVE ops (no partition broadcast needed).
  - Output partials are written fp16; host sums in fp32.
"""

import math

import numpy as np

import concourse.bass as bass
from concourse import bacc
import concourse.mybir as mybir
import concourse.tile as tile
from concourse.bass_utils import run_bass_kernel_spmd

F32 = mybir.dt.float32
F16 = mybir.dt.float16
EXP = mybir.ActivationFunctionType.Exp

B, S, D, HID, H, DH = 2, 2048, 512, 512, 8, 64
NQC = S // 512       # q chunks


def _build(vl0: int, vl1: int):
    """One BIR program, same on all 8 cores. vl0/vl1 = valid lengths
    for the even/odd head of the pair (reference's np.tile quirk)."""
    nc = bacc.Bacc("TRN2", target_bir_lowering=False, debug=False,
                   num_devices=8)
    VL = (vl0, vl1)
    NKT = tuple(max(1, int(math.ceil(v / 128))) for v in VL)
    KMAX = max(NKT) * 128
    NKTM = max(NKT)

    qT_d = nc.dram_tensor("qT", [128, 4, S], F16, kind="ExternalInput").ap()
    kT_d = nc.dram_tensor("kT", [128, 4, KMAX], F16, kind="ExternalInput").ap()
    vT_d = nc.dram_tensor("vT", [128, 4, KMAX], F16, kind="ExternalInput").ap()
    w_d = nc.dram_tensor("w", [128, 12, 128], F16, kind="ExternalInput").ap()
    wo_d = nc.dram_tensor("wo", [128, 512], F16, kind="ExternalInput").ap()
    out_d = nc.dram_tensor("out", [128, S // 128, 512], F16,
                           kind="ExternalOutput").ap()

    with tile.TileContext(nc) as tc:
        with (
            tc.tile_pool(name="consts", bufs=1) as consts,
            tc.tile_pool(name="inputs", bufs=1) as inputs,
            tc.tile_pool(name="work", bufs=1) as work,
            tc.tile_pool(name="exps", bufs=4) as exps,
            tc.tile_pool(name="small", bufs=4) as small,
            tc.tile_pool(name="outs", bufs=4) as outs,
            tc.tile_pool(name="pst", bufs=2, space="PSUM") as pst,
            tc.tile_pool(name="pssc", bufs=2, space="PSUM") as pssc,
            tc.tile_pool(name="pspv", bufs=2, space="PSUM") as pspv,
        ):
            # PE warm-up: the HAM clock gate keeps an idle PE at 1.2 GHz
            # and needs ~3.4us of sustained activity to unthrottle. Burn
            # dummy matmuls on a zeroed scratch tile while inputs stream
            # in, so real matmuls start at 2.4 GHz.
            scratch = consts.tile([128, 512], F16)
            nc.vector.memset(scratch[:], 0.0)
            for _ in range(4):
                pw = pst.tile([128, 512], F32, tag="t", name="pwu")
                nc.tensor.matmul(pw[:], scratch[:, 0:128], scratch[:],
                                 start=True, stop=True)

            # weights as one packed DMA: rows 0-3 = Wk dt-chunks,
            # 4-7 = Wq, 8-11 = Wv
            w_s = consts.tile([128, 12, 128], F16)
            wo_s = consts.tile([128, 512], F16)
            nc.sync.dma_start(w_s[:], w_d[:])

            # inputs split into first-512 + rest, issue spread across the
            # sync/gpsimd/scalar queues (DMA issue costs ~0.6us each on a
            # single queue; serial issue was the old 30us startup stall)
            qT_in = inputs.tile([128, 4, S], F16)
            kT_in = inputs.tile([128, 4, KMAX], F16)
            vT_in = inputs.tile([128, 4, KMAX], F16)
            # DMA issue costs ~0.7us each serially on the issuing queue,
            # so inputs are partition-major in DRAM and move as ONE
            # dma_start per (tensor, first-chunk/rest), split across the
            # two HWDGE queues (sync + scalar).
            KR = min(512, KMAX)
            nc.sync.dma_start(kT_in[:, :, 0:KR], kT_d[:, :, 0:KR])
            nc.scalar.dma_start(vT_in[:, :, 0:KR], vT_d[:, :, 0:KR])
            nc.sync.dma_start(qT_in[:, :, 0:512], qT_d[:, :, 0:512])
            if KMAX > KR:
                nc.sync.dma_start(kT_in[:, :, KR:], kT_d[:, :, KR:])
                nc.scalar.dma_start(vT_in[:, :, KR:], vT_d[:, :, KR:])
            nc.sync.dma_start(qT_in[:, :, 512:], qT_d[:, :, 512:])
            nc.sync.dma_start(wo_s[:], wo_d[:])

            # ---- projections (emitted just-in-time into the attention
            # stream, so the PE FIFO never waits on late DMA chunks) ----
            qT = work.tile([128, S], F16)      # [2*64 rows, q]
            kT = work.tile([128, KMAX], F16)   # rows l*64.., keys

            # v_aug [128 keys, NKTM, 2, 64+64]: cols 64:128 = ones ->
            # the PV matmul emits the softmax denominator replicated on
            # 64 partitions. Invalid key rows (>= vl, boundary tile) are
            # zeroed INCLUDING the ones block: they contribute 0 to
            # numerator+denominator, which IS the masked softmax — no
            # exp bias needed.
            vaug = work.tile([128, NKTM, 2, 128], F16)
            valid = [VL[l] - (NKT[l] - 1) * 128 for l in range(2)]
            for l in range(2):
                nk, va = NKT[l], valid[l]
                if nk > 1:
                    nc.gpsimd.memset(vaug[:, 0:nk - 1, l, 64:128], 1.0)
                if va < 128:
                    nc.gpsimd.memset(vaug[:, nk - 1, l, :], 0.0)
                nc.gpsimd.memset(vaug[0:va, nk - 1, l, 64:128], 1.0)

            def kproj_chunk(l, pos):
                ncols = min(512, NKT[l] * 128 - pos)
                ps = pst.tile([128, 512], F32, tag="t", name="pkv")
                o = ps[l * 64:(l + 1) * 64, :ncols]
                for dt in range(4):
                    nc.tensor.matmul(
                        o, w_s[:, dt, l * 64:(l + 1) * 64],
                        kT_in[:, dt, pos:pos + ncols],
                        start=(dt == 0), stop=(dt == 3))
                nc.scalar.copy(
                    kT[l * 64:(l + 1) * 64, pos:pos + ncols], o)

            def qproj_chunk(c):
                ps = pst.tile([128, 512], F32, tag="t", name="pq")
                for dt in range(4):
                    nc.tensor.matmul(ps[:], w_s[:, 4 + dt],
                                     qT_in[:, dt, c * 512:(c + 1) * 512],
                                     start=(dt == 0), stop=(dt == 3))
                nc.vector.tensor_copy(qT[:, c * 512:(c + 1) * 512], ps[:])

            def vproj_one(kt):
                heads = [l for l in range(2) if kt < NKT[l]]
                pv_ = pst.tile([128, 512], F32, tag="t", name="pvp")
                if len(heads) == 2:
                    o, cols = pv_[:, 0:128], slice(0, 128)
                else:
                    lo = heads[0]
                    o = pv_[:, lo * 64:(lo + 1) * 64]
                    cols = slice(lo * 64, (lo + 1) * 64)
                for dt in range(4):
                    nc.tensor.matmul(
                        o, vT_in[:, dt, kt * 128:(kt + 1) * 128],
                        w_s[:, 8 + dt, cols],
                        start=(dt == 0), stop=(dt == 3))
                for lo in heads:
                    va = valid[lo] if kt == NKT[lo] - 1 else 128
                    nc.vector.tensor_copy(
                        vaug[0:va, kt, lo, 0:64],
                        pv_[0:va, lo * 64:(lo + 1) * 64])

            emitted_k = [set(), set()]
            emitted_v = set()
            emitted_q = set()

            def need(l, kts, qc):
                for kt in kts:
                    c = kt // 4
                    if c not in emitted_k[l]:
                        emitted_k[l].add(c)
                        kproj_chunk(l, c * 512)
                    if kt not in emitted_v:
                        emitted_v.add(kt)
                        vproj_one(kt)
                if qc not in emitted_q:
                    emitted_q.add(qc)
                    qproj_chunk(qc)

            # ---- attention per (q-chunk, head) ----
            outT = work.tile([128, S], F16)

            def wo_stage(qc):
                so = outs.tile([128, 4, 512], F16, tag="so", name="so")
                for j, qt in enumerate(range(qc * 4, (qc + 1) * 4)):
                    po = pst.tile([128, 512], F32, tag="t", name="po")
                    nc.tensor.matmul(po[:], outT[:, qt * 128:(qt + 1) * 128],
                                     wo_s[:], start=True, stop=True)
                    nc.vector.tensor_copy(so[:, j], po[:])
                nc.gpsimd.dma_start(out_d[:, qc * 4:(qc + 1) * 4], so[:])

            for qc in range(NQC):
                for l in range(2):
                    nkt = NKT[l]
                    qs = qT[l * 64:(l + 1) * 64, qc * 512:(qc + 1) * 512]
                    pv = pspv.tile([128, 512], F32, tag="pv", name="pv")
                    for kt0 in range(0, nkt, 2):
                        kts = [kt for kt in (kt0, kt0 + 1) if kt < nkt]
                        need(l, kts, qc)
                        sc = pssc.tile([128, 1024], F32, tag="sc", name="sc")
                        es = exps.tile([128, 1024], F16, tag="es", name="es")
                        for i, kt in enumerate(kts):
                            nc.tensor.matmul(
                                sc[:, i * 512:(i + 1) * 512],
                                kT[l * 64:(l + 1) * 64,
                                   kt * 128:(kt + 1) * 128],
                                qs, start=True, stop=True)
                        n = len(kts) * 512
                        nc.scalar.activation(es[:, 0:n], sc[:, 0:n], EXP,
                                             scale=0.125)
                        for i, kt in enumerate(kts):
                            nc.tensor.matmul(
                                pv[:], vaug[:, kt, l, :],
                                es[:, i * 512:(i + 1) * 512],
                                start=(kt == 0), stop=(kt == nkt - 1))
                        if (l == 0 and qc > 0
                                and kt0 == (2 if nkt > 2 else 0)):
                            wo_stage(qc - 1)   # pipelined: PE never stalls
                    den = small.tile([64, 512], F32, tag="den", name="den")
                    nc.vector.tensor_copy(den[:], pv[64:128, :])
                    rec = small.tile([64, 512], F32, tag="rec", name="rec")
                    nc.vector.reciprocal_approx_fast(rec[:], den[:])
                    nc.vector.tensor_mul(
                        outT[l * 64:(l + 1) * 64, qc * 512:(qc + 1) * 512],
                        pv[0:64, :], rec[:])

            wo_stage(NQC - 1)
    nc.compile()
    return nc


_CACHE: dict = {}


def kernel(query, key, value, Wq, Wk, Wv, Wo, valid_length):
    query = np.asarray(query); key = np.asarray(key); value = np.asarray(value)
    Wq = np.asarray(Wq, np.float32); Wk = np.asarray(Wk, np.float32)
    Wv = np.asarray(Wv, np.float32); Wo = np.asarray(Wo, np.float32)
    vl = np.asarray(valid_length).astype(np.int64)
    # head h is masked with vl[h % 2] (reference's np.tile quirk)
    key_ = (int(vl[0]), int(vl[1]))
    if key_ not in _CACHE:
        _CACHE[key_] = _build(*key_)
    nc = _CACHE[key_]
    nkt = [max(1, int(math.ceil(int(vl[l]) / 128))) for l in range(2)]
    KMAX = max(nkt) * 128

    f16 = np.float16
    pm = lambda a: np.ascontiguousarray(
        a.T.astype(f16).reshape(4, 128, -1).transpose(1, 0, 2))
    qT_b = [pm(query[b]) for b in range(B)]
    kT_b = [pm(key[b, :KMAX]) for b in range(B)]
    vT_b = [pm(value[b, :KMAX]) for b in range(B)]
    w_p = [np.ascontiguousarray(np.concatenate([
               W[:, p * 128:(p + 1) * 128].astype(f16).reshape(4, 128, 128)
               for W in (Wk, Wq, Wv)], axis=0).transpose(1, 0, 2))
           for p in range(4)]
    wo_p = [np.ascontiguousarray(Wo[p * 128:(p + 1) * 128]).astype(f16)
            for p in range(4)]

    in_maps = []
    for c in range(8):
        b, p = c // 4, c % 4
        in_maps.append({
            "qT": qT_b[b], "kT": kT_b[b], "vT": vT_b[b],
            "w": w_p[p], "wo": wo_p[p],
        })

    import os
    trace = os.environ.get("BASS_KTRACE", "0") == "1"
    kw = dict(trace=True, trace_cores=list(range(8))) if trace else {}
    res = run_bass_kernel_spmd(nc, in_maps, core_ids=list(range(8)), **kw)
    kernel.last_results = res
    out = np.zeros((B, S, HID), np.float32)
    for c in range(8):
        b, p = c // 4, c % 4
        out[b] += res.results[c]["out"].transpose(1, 0, 2).reshape(
            S, HID).astype(np.float32)
    return out


# revision 20
# speedup vs baseline: 1.0188x; 1.0188x over previous
"""Trainium2 Bass kernel for masked multi-head attention (8-core SPMD).

Problem: B=2, S=2048, d_in=hid=512, H=8 heads (dh=64), fp32 in/out.
Reference quirk: the mask uses np.tile(valid_length, H), so scores row
i = b*H + h is masked with valid_length[(b*H + h) % 2] = vl[h % 2] —
the mask depends on HEAD PARITY, not batch. Even heads use vl[0], odd
heads vl[1], in both batches.

Sharding (8 cores): core c = (batch b = c//4, head-pair p = c%4).
Each core computes heads {2p, 2p+1} of batch b over the full 2048
queries, producing its partial output [2048, 512] (through its 128
rows of Wo). Host sums the 4 pair-partials per batch (pure unshard).
Load is balanced by construction: every core has one even (long mask)
and one odd (short mask) head.

Perf design (fp16 data path, fp32 PSUM accumulation):
  - All operands stream through the PE at 1 cycle/row (fp32 is 4x
    slower); host pre-converts inputs to fp16, halving HBM traffic.
  - V is projected TRANSPOSED (stationary = 128-key tile of value^T,
    moving = Wv columns) so v_aug [keys, dh] needs no PE transposes.
  - Masking is folded into v_aug: invalid key rows (>= vl, boundary
    tile only) are zeroed INCLUDING the ones-column, so they add 0 to
    both the PV numerator and the softmax denominator — no exp bias,
    which lets exp run on [128, 1024] paired tiles (halves ACT
    instruction overhead). exp(score/8) <= e^~9, safe in fp16.
  - The vaug ones-block is 64 columns wide, so the PV matmul emits
    the softmax denominator already replicated across 64 PSUM
    partitions: normalization is copy + reciprocal_approx_fast +
    multiply, all plain DVE ops (no partition broadcast needed).
  - Output partials are written fp16; host sums in fp32.
"""

import math

import numpy as np

import concourse.bass as bass
from concourse import bacc
import concourse.mybir as mybir
import concourse.tile as tile
from concourse.bass_utils import run_bass_kernel_spmd

F32 = mybir.dt.float32
F16 = mybir.dt.float16
EXP = mybir.ActivationFunctionType.Exp

B, S, D, HID, H, DH = 2, 2048, 512, 512, 8, 64
NQC = S // 512       # q chunks


def _build(vl0: int, vl1: int):
    """One BIR program, same on all 8 cores. vl0/vl1 = valid lengths
    for the even/odd head of the pair (reference's np.tile quirk)."""
    nc = bacc.Bacc("TRN2", target_bir_lowering=False, debug=False,
                   num_devices=8)
    VL = (vl0, vl1)
    NKT = tuple(max(1, int(math.ceil(v / 128))) for v in VL)
    KMAX = max(NKT) * 128
    NKTM = max(NKT)

    qT_d = nc.dram_tensor("qT", [128, 4, S], F16, kind="ExternalInput").ap()
    kT_d = nc.dram_tensor("kT", [128, 4, KMAX], F16, kind="ExternalInput").ap()
    vT_d = nc.dram_tensor("vT", [128, 4, KMAX], F16, kind="ExternalInput").ap()
    w_d = nc.dram_tensor("w", [128, 12, 128], F16, kind="ExternalInput").ap()
    wo_d = nc.dram_tensor("wo", [128, 512], F16, kind="ExternalInput").ap()
    out_d = nc.dram_tensor("out", [128, S // 128, 512], F16,
                           kind="ExternalOutput").ap()

    with tile.TileContext(nc) as tc:
        with (
            tc.tile_pool(name="consts", bufs=1) as consts,
            tc.tile_pool(name="inputs", bufs=1) as inputs,
            tc.tile_pool(name="work", bufs=1) as work,
            tc.tile_pool(name="exps", bufs=4) as exps,
            tc.tile_pool(name="small", bufs=4) as small,
            tc.tile_pool(name="outs", bufs=4) as outs,
            tc.tile_pool(name="pst", bufs=2, space="PSUM") as pst,
            tc.tile_pool(name="pssc", bufs=2, space="PSUM") as pssc,
            tc.tile_pool(name="pspv", bufs=2, space="PSUM") as pspv,
        ):
            # PE warm-up: the HAM clock gate keeps an idle PE at 1.2 GHz
            # and needs ~3.4us of sustained activity to unthrottle. Burn
            # dummy matmuls on a zeroed scratch tile while inputs stream
            # in, so real matmuls start at 2.4 GHz.
            scratch = consts.tile([128, 512], F16)
            nc.vector.memset(scratch[:], 0.0)
            for _ in range(4):
                pw = pst.tile([128, 512], F32, tag="t", name="pwu")
                nc.tensor.matmul(pw[:], scratch[:, 0:128], scratch[:],
                                 start=True, stop=True)

            # weights as one packed DMA: rows 0-3 = Wk dt-chunks,
            # 4-7 = Wq, 8-11 = Wv
            w_s = consts.tile([128, 12, 128], F16)
            wo_s = consts.tile([128, 512], F16)
            nc.sync.dma_start(w_s[:], w_d[:])

            # inputs split into first-512 + rest, issue spread across the
            # sync/gpsimd/scalar queues (DMA issue costs ~0.6us each on a
            # single queue; serial issue was the old 30us startup stall)
            qT_in = inputs.tile([128, 4, S], F16)
            kT_in = inputs.tile([128, 4, KMAX], F16)
            vT_in = inputs.tile([128, 4, KMAX], F16)
            # DMA issue costs ~0.7us each serially on the issuing queue,
            # so inputs are partition-major in DRAM and move as ONE
            # dma_start per (tensor, first-chunk/rest), split across the
            # two HWDGE queues (sync + scalar).
            KR = min(512, KMAX)
            nc.sync.dma_start(kT_in[:, :, 0:KR], kT_d[:, :, 0:KR])
            nc.scalar.dma_start(vT_in[:, :, 0:KR], vT_d[:, :, 0:KR])
            nc.sync.dma_start(qT_in[:, :, 0:512], qT_d[:, :, 0:512])
            for pos in range(KR, KMAX, 512):
                pe_ = min(pos + 512, KMAX)
                nc.sync.dma_start(kT_in[:, :, pos:pe_], kT_d[:, :, pos:pe_])
                nc.scalar.dma_start(vT_in[:, :, pos:pe_], vT_d[:, :, pos:pe_])
            nc.sync.dma_start(qT_in[:, :, 512:], qT_d[:, :, 512:])
            nc.sync.dma_start(wo_s[:], wo_d[:])

            # ---- projections (emitted just-in-time into the attention
            # stream, so the PE FIFO never waits on late DMA chunks) ----
            qT = work.tile([128, S], F16)      # [2*64 rows, q]
            kT = work.tile([128, KMAX], F16)   # rows l*64.., keys

            # v_aug [128 keys, NKTM, 2, 64+64]: cols 64:128 = ones ->
            # the PV matmul emits the softmax denominator replicated on
            # 64 partitions. Invalid key rows (>= vl, boundary tile) are
            # zeroed INCLUDING the ones block: they contribute 0 to
            # numerator+denominator, which IS the masked softmax — no
            # exp bias needed.
            vaug = work.tile([128, NKTM, 2, 128], F16)
            valid = [VL[l] - (NKT[l] - 1) * 128 for l in range(2)]
            for l in range(2):
                nk, va = NKT[l], valid[l]
                if nk > 1:
                    nc.gpsimd.memset(vaug[:, 0:nk - 1, l, 64:128], 1.0)
                if va < 128:
                    nc.gpsimd.memset(vaug[:, nk - 1, l, :], 0.0)
                nc.gpsimd.memset(vaug[0:va, nk - 1, l, 64:128], 1.0)

            def kproj_chunk(l, pos):
                ncols = min(512, NKT[l] * 128 - pos)
                ps = pst.tile([128, 512], F32, tag="t", name="pkv")
                o = ps[l * 64:(l + 1) * 64, :ncols]
                for dt in range(4):
                    nc.tensor.matmul(
                        o, w_s[:, dt, l * 64:(l + 1) * 64],
                        kT_in[:, dt, pos:pos + ncols],
                        start=(dt == 0), stop=(dt == 3))
                nc.scalar.copy(
                    kT[l * 64:(l + 1) * 64, pos:pos + ncols], o)

            def qproj_chunk(c):
                ps = pst.tile([128, 512], F32, tag="t", name="pq")
                for dt in range(4):
                    nc.tensor.matmul(ps[:], w_s[:, 4 + dt],
                                     qT_in[:, dt, c * 512:(c + 1) * 512],
                                     start=(dt == 0), stop=(dt == 3))
                nc.vector.tensor_copy(qT[:, c * 512:(c + 1) * 512], ps[:])

            def vproj_one(kt):
                heads = [l for l in range(2) if kt < NKT[l]]
                pv_ = pst.tile([128, 512], F32, tag="t", name="pvp")
                if len(heads) == 2:
                    o, cols = pv_[:, 0:128], slice(0, 128)
                else:
                    lo = heads[0]
                    o = pv_[:, lo * 64:(lo + 1) * 64]
                    cols = slice(lo * 64, (lo + 1) * 64)
                for dt in range(4):
                    nc.tensor.matmul(
                        o, vT_in[:, dt, kt * 128:(kt + 1) * 128],
                        w_s[:, 8 + dt, cols],
                        start=(dt == 0), stop=(dt == 3))
                for lo in heads:
                    va = valid[lo] if kt == NKT[lo] - 1 else 128
                    nc.vector.tensor_copy(
                        vaug[0:va, kt, lo, 0:64],
                        pv_[0:va, lo * 64:(lo + 1) * 64])

            emitted_k = [set(), set()]
            emitted_v = set()
            emitted_q = set()

            def need(l, kts, qc):
                for kt in kts:
                    c = kt // 4
                    if c not in emitted_k[l]:
                        emitted_k[l].add(c)
                        kproj_chunk(l, c * 512)
                    if kt not in emitted_v:
                        emitted_v.add(kt)
                        vproj_one(kt)
                if qc not in emitted_q:
                    emitted_q.add(qc)
                    qproj_chunk(qc)

            # ---- attention per (q-chunk, head) ----
            outT = work.tile([128, S], F16)

            def wo_stage(qc):
                so = outs.tile([128, 4, 512], F16, tag="so", name="so")
                for j, qt in enumerate(range(qc * 4, (qc + 1) * 4)):
                    po = pst.tile([128, 512], F32, tag="t", name="po")
                    nc.tensor.matmul(po[:], outT[:, qt * 128:(qt + 1) * 128],
                                     wo_s[:], start=True, stop=True)
                    nc.vector.tensor_copy(so[:, j], po[:])
                nc.gpsimd.dma_start(out_d[:, qc * 4:(qc + 1) * 4], so[:])

            horder = sorted((0, 1), key=lambda x: NKT[x])
            for qc in range(NQC):
                for l in horder:
                    nkt = NKT[l]
                    qs = qT[l * 64:(l + 1) * 64, qc * 512:(qc + 1) * 512]
                    pv = pspv.tile([128, 512], F32, tag="pv", name="pv")
                    for kt0 in range(0, nkt, 2):
                        kts = [kt for kt in (kt0, kt0 + 1) if kt < nkt]
                        need(l, kts, qc)
                        sc = pssc.tile([128, 1024], F32, tag="sc", name="sc")
                        es = exps.tile([128, 1024], F16, tag="es", name="es")
                        for i, kt in enumerate(kts):
                            nc.tensor.matmul(
                                sc[:, i * 512:(i + 1) * 512],
                                kT[l * 64:(l + 1) * 64,
                                   kt * 128:(kt + 1) * 128],
                                qs, start=True, stop=True)
                        n = len(kts) * 512
                        nc.scalar.activation(es[:, 0:n], sc[:, 0:n], EXP,
                                             scale=0.125)
                        for i, kt in enumerate(kts):
                            nc.tensor.matmul(
                                pv[:], vaug[:, kt, l, :],
                                es[:, i * 512:(i + 1) * 512],
                                start=(kt == 0), stop=(kt == nkt - 1))
                        if (l == horder[1] and qc > 0
                                and kt0 == (2 if nkt > 2 else 0)):
                            wo_stage(qc - 1)   # pipelined: PE never stalls
                    den = small.tile([64, 512], F32, tag="den", name="den")
                    nc.vector.tensor_copy(den[:], pv[64:128, :])
                    rec = small.tile([64, 512], F32, tag="rec", name="rec")
                    nc.vector.reciprocal_approx_fast(rec[:], den[:])
                    nc.vector.tensor_mul(
                        outT[l * 64:(l + 1) * 64, qc * 512:(qc + 1) * 512],
                        pv[0:64, :], rec[:])

            wo_stage(NQC - 1)
    nc.compile()
    return nc


_CACHE: dict = {}


def kernel(query, key, value, Wq, Wk, Wv, Wo, valid_length):
    query = np.asarray(query); key = np.asarray(key); value = np.asarray(value)
    Wq = np.asarray(Wq, np.float32); Wk = np.asarray(Wk, np.float32)
    Wv = np.asarray(Wv, np.float32); Wo = np.asarray(Wo, np.float32)
    vl = np.asarray(valid_length).astype(np.int64)
    # head h is masked with vl[h % 2] (reference's np.tile quirk)
    key_ = (int(vl[0]), int(vl[1]))
    if key_ not in _CACHE:
        _CACHE[key_] = _build(*key_)
    nc = _CACHE[key_]
    nkt = [max(1, int(math.ceil(int(vl[l]) / 128))) for l in range(2)]
    KMAX = max(nkt) * 128

    f16 = np.float16
    pm = lambda a: np.ascontiguousarray(
        a.T.astype(f16).reshape(4, 128, -1).transpose(1, 0, 2))
    qT_b = [pm(query[b]) for b in range(B)]
    kT_b = [pm(key[b, :KMAX]) for b in range(B)]
    vT_b = [pm(value[b, :KMAX]) for b in range(B)]
    w_p = [np.ascontiguousarray(np.concatenate([
               W[:, p * 128:(p + 1) * 128].astype(f16).reshape(4, 128, 128)
               for W in (Wk, Wq, Wv)], axis=0).transpose(1, 0, 2))
           for p in range(4)]
    wo_p = [np.ascontiguousarray(Wo[p * 128:(p + 1) * 128]).astype(f16)
            for p in range(4)]

    in_maps = []
    for c in range(8):
        b, p = c // 4, c % 4
        in_maps.append({
            "qT": qT_b[b], "kT": kT_b[b], "vT": vT_b[b],
            "w": w_p[p], "wo": wo_p[p],
        })

    import os
    trace = os.environ.get("BASS_KTRACE", "0") == "1"
    kw = dict(trace=True, trace_cores=list(range(8))) if trace else {}
    res = run_bass_kernel_spmd(nc, in_maps, core_ids=list(range(8)), **kw)
    kernel.last_results = res
    out = np.zeros((B, S, HID), np.float32)
    for c in range(8):
        b, p = c // 4, c % 4
        out[b] += res.results[c]["out"].transpose(1, 0, 2).reshape(
            S, HID).astype(np.float32)
    return out
